# revision 12
# baseline (speedup 1.0000x reference)
"""Batch-assign-probability (VQ codebook softmax) kernel for 8 Trainium2 cores.

Math: for each valid row x (D=512), over K=256 centers c_k:
    softmax_k(-||x - c_k||^2) == softmax_k(2 x.c_k - ||c_k||^2)
(the ||x||^2 term is constant over k and cancels in softmax).

v2 design (vs the 3-pass bf16 baseline):
  - fp16 single-pass main matmul: x_fp16 @ (2c)_fp16^T, full PE rate. fp16's
    10+1 mantissa bits put the logit error at ~7e-3 rms; a single extra
    correction pass kills the centers-side quantization error:
        corr = x_e5m2 @ ((2c) - fp16(2c))_e5m2
    run as 2 fp8 DoubleRow matmuls (2x rate, 256-contraction per inst).
    End-to-end max abs output err ~9.8e-3 vs the 2e-2 gate (simulated).
  - bias -||c||^2 folded in as one fp16 matmul against a 3-level fp16 split.
  - exp(l - rowmax) on ACT writes fp16 directly; the softmax NORMALIZATION
    (multiply by exp-bias residual and divide by the row sum) happens on the
    HOST, removing the accum-read + reciprocal + broadcast-multiply chain
    from the device critical path.
  - All x traffic for a row group rides ONE byte-packed DMA (fp16 hi bytes
    then fp8 bytes per partition), bitcast back into typed views on SBUF:
    3.1MB/core of x instead of 4.2MB, 13 descriptors total on the sync ring.
  - Device per 128-row tile: 4 fp16 + 2 fp8-DR + 1 bias matmul -> PSUM,
    DVE reduce_max(negate) per pair, ACT exp -> fp16 out tile, grouped
    out-DMA. Scalar engine runs ONLY the 16 exps.
"""

import numpy as np
import ml_dtypes

import concourse.bacc as bacc
import concourse.tile as tile
from concourse import mybir
from concourse.bass_utils import run_bass_kernel_spmd

B, T, W, C, K = 16, 2048, 512, 1, 256
VALID_T = 1024
D = W * C                       # 512
N_CORES = 8
B_PER_CORE = B // N_CORES       # 2
ROWS = B_PER_CORE * VALID_T     # 2048 rows per core
P = 128
DC = D // P                     # 4 contraction chunks
GROUPS = [128, 256, 384, 384, 512, 384]   # rows per x/out DMA group
N_WARM_MM = 5                   # dummy matmuls to lift the PE HAM clock-gate
assert sum(GROUPS) == ROWS
assert all(g % P == 0 for g in GROUPS)

# The -||c||^2 bias rides the fp8 DoubleRow correction pass: x8 contraction
# rows 509-511 are hardwired to 1.0 and ctl8 rows 509-511 carry a 3-level
# e5m2 split of the bias (residual <= 0.25, compensated exactly by the host
# softmax weight). No separate bias matmul.
XB = 12                         # x bytes per row element-slot per partition
CONST_B = 2048 + 1024

F16 = np.float16
E5M2 = ml_dtypes.float8_e5m2

_CACHE: dict = {}


def _build_bass():
    f32 = mybir.dt.float32
    f16 = mybir.dt.float16
    f8 = mybir.dt.float8e5
    u8 = mybir.dt.uint8
    nc = bacc.Bacc()
    xp = nc.declare_dram_parameter("xp", [P * XB * ROWS], u8, isOutput=False)
    cp = nc.declare_dram_parameter("cp", [P * CONST_B], u8, isOutput=False)
    # partition-major out layout: per-partition contiguous 512B runs per
    # subtile, so out DMAs emit few large descriptors; host un-permutes.
    out = nc.declare_dram_parameter("out", [P, ROWS // P, K], f16,
                                    isOutput=True)
    out_v = out                                          # [128, 16, 256]

    with tile.TileContext(nc) as tc:
        with (
            tc.tile_pool(name="singles", bufs=1) as singles,
            tc.tile_pool(name="xpool", bufs=1) as xpool,
            tc.tile_pool(name="opool", bufs=3) as opool,
            tc.tile_pool(name="small", bufs=8) as small,
            tc.tile_pool(name="psum", bufs=7, space="PSUM") as psum,
            tc.tile_pool(name="psum_warm", bufs=1, space="PSUM") as psum_warm,
        ):
            # constants first on the sync ring: they gate the first matmul
            # together with group 0 (the scalar ring would park them behind
            # the 1.3us ACT table load)
            csb = singles.tile([P, CONST_B], u8)
            nc.sync.dma_start(out=csb[:], in_=cp.rearrange("(p b) -> p b", p=P))
            c_ap = csb[:]
            o = 0
            ct_v = c_ap[:, o:o + 2 * DC * K].bitcast(f16).rearrange(
                "p (c k) -> p c k", c=DC)
            o += 2 * DC * K
            ctl_v = c_ap[:, o:o + DC * K].bitcast(f8).rearrange(
                "p (j i k) -> p j i k", j=2, i=2)

            xgs = []
            xoff = 0
            for g, R in enumerate(GROUPS):
                n = P * XB * R
                xg = xpool.tile([P, XB * R], u8, tag=f"xg{g}")
                nc.sync.dma_start(
                    out=xg[:],
                    in_=xp[xoff:xoff + n].rearrange("(p b) -> p b", p=P))
                xoff += n
                ap = xg[:]
                xh_v = ap[:, :8 * R].bitcast(f16).rearrange(
                    "p (c r) -> p c r", c=DC)
                x8_v = ap[:, 8 * R:].bitcast(f8).rearrange(
                    "p (j i r) -> p j i r", j=2, i=2)
                xgs.append((xh_v, x8_v))

            # PE warm-up: dummy matmuls keep the PE busy through the HAM
            # activity window while the first x DMA lands.
            warm_sb = singles.tile([P, 512], f16)
            nc.gpsimd.memset(warm_sb[:], 0.0)
            warm_ps = psum_warm.tile([P, 512], f32, tag="warm")
            for _ in range(N_WARM_MM):
                nc.tensor.matmul(
                    warm_ps[:], lhsT=warm_sb[:, :P], rhs=warm_sb[:],
                    start=True, stop=True,
                )

            t0 = 0  # running 128-row tile index
            for g, R in enumerate(GROUPS):
                xh_v, x8_v = xgs[g]
                subtiles = R // P
                og = opool.tile([P, subtiles, K], f16, tag="og")
                last_g = g == len(GROUPS) - 1
                for s0 in range(0, subtiles, 2):
                    pair = min(2, subtiles - s0)
                    ps = psum.tile([P, pair, K], f32, tag="ps")
                    for j in range(pair):
                        s = s0 + j
                        rsl = slice(s * P, (s + 1) * P)
                        for c in range(DC):
                            nc.tensor.matmul(
                                ps[:, j, :],
                                lhsT=xh_v[:, c, rsl],
                                rhs=ct_v[:, c, :],
                                start=(c == 0),
                                stop=False,
                            )
                        for jd in range(2):
                            nc.tensor.matmul(
                                ps[:, j, :],
                                lhsT=x8_v[:, jd, :, rsl],
                                rhs=ctl_v[:, jd],
                                start=False,
                                stop=(jd == 1),
                                perf_mode=mybir.MatmulPerfMode.DoubleRow,
                            )
                    negm = small.tile([P, pair], f32, tag="negm")
                    nc.vector.reduce_max(
                        out=negm[:], in_=ps[:], axis=mybir.AxisListType.X,
                        negate=True,
                    )
                    for j in range(pair):
                        nc.scalar.activation(
                            out=og[:, s0 + j, :],
                            in_=ps[:, j, :],
                            func=mybir.ActivationFunctionType.Exp,
                            bias=negm[:, j:j + 1],
                            scale=1.0,
                        )
                    # per-pair out DMA; the final group issues from the
                    # (idle-at-tail) scalar queue so it isn't stuck behind
                    # the sync ring.
                    eng = nc.scalar if last_g else nc.sync
                    eng.dma_start(
                        out=out_v[:, t0 + s0:t0 + s0 + pair, :],
                        in_=og[:, s0:s0 + pair, :],
                    )
                t0 += subtiles
    nc.finalize()
    return nc


def get_nc():
    if "nc" not in _CACHE:
        _CACHE["nc"] = _build_bass()
    return _CACHE["nc"]


def prep_inputs(y_pred: np.ndarray, mask: np.ndarray, centers: np.ndarray):
    """Host-side prep: valid-timestep slice, per-core transpose, fp16/fp8
    packing (one contiguous byte blob per DMA), bias splits, host-side
    softmax weight table."""
    x = np.ascontiguousarray(y_pred.reshape(B, T, D))
    masktime = np.asarray(mask).reshape(B, T, D)[0, :, 0]
    valid_idx = np.nonzero(masktime == 0)[0][:VALID_T]
    assert valid_idx.shape[0] == VALID_T
    if valid_idx[0] == 0 and valid_idx[-1] == VALID_T - 1:
        xv = x[:, :VALID_T]                    # [B, VALID_T, D]
    else:
        xv = x[:, valid_idx]

    centers64 = np.asarray(centers, dtype=np.float64)
    ct = (2.0 * centers64).T                                # [D, K]
    cth = ct.astype(F16)
    negc2 = -(centers64 ** 2).sum(axis=1)                   # [K]
    # 3-level e5m2 bias cascade, carried in ctl8 contraction rows 509-511
    b1 = negc2.astype(E5M2)
    r1 = negc2 - b1.astype(np.float64)
    b2 = r1.astype(E5M2)
    b3 = (r1 - b2.astype(np.float64)).astype(E5M2)

    # host-side per-center softmax weight: exact residual of the device bias
    lw = negc2 - (b1.astype(np.float64) + b2.astype(np.float64)
                  + b3.astype(np.float64))
    w_host = np.exp(lw - lw.max()).astype(np.float32)       # [K], ~1.0
    _CACHE["w_host"] = w_host

    ctl = (ct - cth.astype(np.float64)).astype(E5M2)        # [D, K]
    ctl[509], ctl[510], ctl[511] = b1, b2, b3
    # const pack [P, CONST_B] bytes: ct | ctl
    parts = [
        np.ascontiguousarray(
            cth.reshape(DC, P, K).transpose(1, 0, 2)
        ).reshape(P, DC * K).view(np.uint8),
        np.ascontiguousarray(
            ctl.reshape(2, 2, P, K).transpose(2, 0, 1, 3)
        ).reshape(P, DC * K).view(np.uint8),
    ]
    cp = np.ascontiguousarray(np.concatenate(parts, axis=1))
    assert cp.shape == (P, CONST_B)
    cp = cp.ravel()

    in_maps = []
    for core in range(N_CORES):
        xc = xv[core * B_PER_CORE:(core + 1) * B_PER_CORE].reshape(ROWS, D)
        xT = np.ascontiguousarray(xc.T)                     # [D, ROWS] f32
        xh = xT.astype(F16)
        xh_p = xh.reshape(DC, P, ROWS).transpose(1, 0, 2)   # [P, DC, ROWS]
        x8 = xT.astype(E5M2)
        x8[509:512] = 1.0      # bias contraction rows (pair with ctl 509-511)
        x8_p = x8.reshape(2, 2, P, ROWS).transpose(2, 0, 1, 3)
        blocks = []
        r0 = 0
        for R in GROUPS:
            hb = np.ascontiguousarray(
                xh_p[:, :, r0:r0 + R]).reshape(P, DC * R).view(np.uint8)
            lb = np.ascontiguousarray(
                x8_p[:, :, :, r0:r0 + R]).reshape(P, DC * R).view(np.uint8)
            blocks.append(np.concatenate([hb, lb], axis=1).ravel())
            r0 += R
        xp_core = np.concatenate(blocks)
        assert xp_core.shape[0] == P * XB * ROWS
        in_maps.append({"xp": xp_core, "cp": cp})
    return in_maps


def kernel(y_pred: np.ndarray, mask: np.ndarray, centers: np.ndarray,
           **run_kwargs) -> np.ndarray:
    in_maps = prep_inputs(y_pred, mask, centers)
    nc = get_nc()
    last_err = None
    for _attempt in range(3):
        try:
            res = run_bass_kernel_spmd(nc, in_maps, core_ids=list(range(N_CORES)),
                                       **run_kwargs)
            break
        except Exception as e:  # transient NRT device errors — retry
            last_err = e
    else:
        raise last_err
    _CACHE["last_results"] = res
    e = np.concatenate(
        [np.asarray(r["out"]).transpose(1, 0, 2).reshape(B_PER_CORE, VALID_T, K)
         for r in res.results], axis=0
    ).astype(np.float32)
    ew = e * _CACHE["w_host"]
    out = ew / ew.sum(axis=-1, keepdims=True)
    return out.astype(np.float32, copy=False)


# revision 13
# speedup vs baseline: 1.0069x; 1.0069x over previous
"""Batch-assign-probability (VQ codebook softmax) kernel for 8 Trainium2 cores.

Math: for each valid row x (D=512), over K=256 centers c_k:
    softmax_k(-||x - c_k||^2) == softmax_k(2 x.c_k - ||c_k||^2)
(the ||x||^2 term is constant over k and cancels in softmax).

v2 design (vs the 3-pass bf16 baseline):
  - fp16 single-pass main matmul: x_fp16 @ (2c)_fp16^T, full PE rate. fp16's
    10+1 mantissa bits put the logit error at ~7e-3 rms; a single extra
    correction pass kills the centers-side quantization error:
        corr = x_e5m2 @ ((2c) - fp16(2c))_e5m2
    run as 2 fp8 DoubleRow matmuls (2x rate, 256-contraction per inst).
    End-to-end max abs output err ~9.8e-3 vs the 2e-2 gate (simulated).
  - bias -||c||^2 folded in as one fp16 matmul against a 3-level fp16 split.
  - exp(l - rowmax) on ACT writes fp16 directly; the softmax NORMALIZATION
    (multiply by exp-bias residual and divide by the row sum) happens on the
    HOST, removing the accum-read + reciprocal + broadcast-multiply chain
    from the device critical path.
  - All x traffic for a row group rides ONE byte-packed DMA (fp16 hi bytes
    then fp8 bytes per partition), bitcast back into typed views on SBUF:
    3.1MB/core of x instead of 4.2MB, 13 descriptors total on the sync ring.
  - Device per 128-row tile: 4 fp16 + 2 fp8-DR + 1 bias matmul -> PSUM,
    DVE reduce_max(negate) per pair, ACT exp -> fp16 out tile, grouped
    out-DMA. Scalar engine runs ONLY the 16 exps.
"""

import numpy as np
import ml_dtypes

import concourse.bacc as bacc
import concourse.tile as tile
from concourse import mybir
from concourse.bass_utils import run_bass_kernel_spmd

B, T, W, C, K = 16, 2048, 512, 1, 256
VALID_T = 1024
D = W * C                       # 512
N_CORES = 8
B_PER_CORE = B // N_CORES       # 2
ROWS = B_PER_CORE * VALID_T     # 2048 rows per core
P = 128
DC = D // P                     # 4 contraction chunks
GROUPS = [128, 256, 384, 384, 512, 384]   # rows per x/out DMA group
N_WARM_MM = 5                   # dummy matmuls to lift the PE HAM clock-gate
assert sum(GROUPS) == ROWS
assert all(g % P == 0 for g in GROUPS)

# The -||c||^2 bias rides the fp8 DoubleRow correction pass: x8 contraction
# rows 509-511 are hardwired to 1.0 and ctl8 rows 509-511 carry a 3-level
# e5m2 split of the bias (residual <= 0.25, compensated exactly by the host
# softmax weight). No separate bias matmul.
XB = 12                         # x bytes per row element-slot per partition
CONST_B = 2048 + 1024

F16 = np.float16
E5M2 = ml_dtypes.float8_e5m2

_CACHE: dict = {}


def _build_bass():
    f32 = mybir.dt.float32
    f16 = mybir.dt.float16
    f8 = mybir.dt.float8e5
    u8 = mybir.dt.uint8
    nc = bacc.Bacc()
    xp = nc.declare_dram_parameter("xp", [P * XB * ROWS], u8, isOutput=False)
    cp = nc.declare_dram_parameter("cp", [P * CONST_B], u8, isOutput=False)
    out = nc.declare_dram_parameter("out", [ROWS, K], f16, isOutput=True)
    out_v = out.rearrange("(t p) k -> p t k", p=P)       # [128, 16, 256]

    with tile.TileContext(nc) as tc:
        with (
            tc.tile_pool(name="singles", bufs=1) as singles,
            tc.tile_pool(name="xpool", bufs=1) as xpool,
            tc.tile_pool(name="opool", bufs=3) as opool,
            tc.tile_pool(name="small", bufs=8) as small,
            tc.tile_pool(name="psum", bufs=7, space="PSUM") as psum,
            tc.tile_pool(name="psum_warm", bufs=1, space="PSUM") as psum_warm,
        ):
            # constants first on the sync ring: they gate the first matmul
            # together with group 0 (the scalar ring would park them behind
            # the 1.3us ACT table load)
            csb = singles.tile([P, CONST_B], u8)
            nc.sync.dma_start(out=csb[:], in_=cp.rearrange("(p b) -> p b", p=P))
            c_ap = csb[:]
            o = 0
            ct_v = c_ap[:, o:o + 2 * DC * K].bitcast(f16).rearrange(
                "p (c k) -> p c k", c=DC)
            o += 2 * DC * K
            ctl_v = c_ap[:, o:o + DC * K].bitcast(f8).rearrange(
                "p (j i k) -> p j i k", j=2, i=2)

            xgs = []
            xoff = 0
            for g, R in enumerate(GROUPS):
                n = P * XB * R
                xg = xpool.tile([P, XB * R], u8, tag=f"xg{g}")
                nc.sync.dma_start(
                    out=xg[:],
                    in_=xp[xoff:xoff + n].rearrange("(p b) -> p b", p=P))
                xoff += n
                ap = xg[:]
                xh_v = ap[:, :8 * R].bitcast(f16).rearrange(
                    "p (c r) -> p c r", c=DC)
                x8_v = ap[:, 8 * R:].bitcast(f8).rearrange(
                    "p (j i r) -> p j i r", j=2, i=2)
                xgs.append((xh_v, x8_v))

            # PE warm-up: dummy matmuls keep the PE busy through the HAM
            # activity window while the first x DMA lands.
            warm_sb = singles.tile([P, 512], f16)
            nc.gpsimd.memset(warm_sb[:], 0.0)
            warm_ps = psum_warm.tile([P, 512], f32, tag="warm")
            for _ in range(N_WARM_MM):
                nc.tensor.matmul(
                    warm_ps[:], lhsT=warm_sb[:, :P], rhs=warm_sb[:],
                    start=True, stop=True,
                )

            t0 = 0  # running 128-row tile index
            for g, R in enumerate(GROUPS):
                xh_v, x8_v = xgs[g]
                subtiles = R // P
                og = opool.tile([P, subtiles, K], f16, tag="og")
                last_g = g == len(GROUPS) - 1
                for s0 in range(0, subtiles, 2):
                    pair = min(2, subtiles - s0)
                    ps = psum.tile([P, pair, K], f32, tag="ps")
                    for j in range(pair):
                        s = s0 + j
                        rsl = slice(s * P, (s + 1) * P)
                        for c in range(DC):
                            nc.tensor.matmul(
                                ps[:, j, :],
                                lhsT=xh_v[:, c, rsl],
                                rhs=ct_v[:, c, :],
                                start=(c == 0),
                                stop=False,
                            )
                        for jd in range(2):
                            nc.tensor.matmul(
                                ps[:, j, :],
                                lhsT=x8_v[:, jd, :, rsl],
                                rhs=ctl_v[:, jd],
                                start=False,
                                stop=(jd == 1),
                                perf_mode=mybir.MatmulPerfMode.DoubleRow,
                            )
                    negm = small.tile([P, pair], f32, tag="negm")
                    nc.vector.reduce_max(
                        out=negm[:], in_=ps[:], axis=mybir.AxisListType.X,
                        negate=True,
                    )
                    for j in range(pair):
                        nc.scalar.activation(
                            out=og[:, s0 + j, :],
                            in_=ps[:, j, :],
                            func=mybir.ActivationFunctionType.Exp,
                            bias=negm[:, j:j + 1],
                            scale=1.0,
                        )
                    # per-pair out DMA; the final group issues from the
                    # (idle-at-tail) scalar queue so it isn't stuck behind
                    # the sync ring.
                    eng = nc.scalar if last_g else nc.sync
                    eng.dma_start(
                        out=out_v[:, t0 + s0:t0 + s0 + pair, :],
                        in_=og[:, s0:s0 + pair, :],
                    )
                t0 += subtiles
    nc.finalize()
    return nc


def get_nc():
    if "nc" not in _CACHE:
        _CACHE["nc"] = _build_bass()
    return _CACHE["nc"]


def prep_inputs(y_pred: np.ndarray, mask: np.ndarray, centers: np.ndarray):
    """Host-side prep: valid-timestep slice, per-core transpose, fp16/fp8
    packing (one contiguous byte blob per DMA), bias splits, host-side
    softmax weight table."""
    x = np.ascontiguousarray(y_pred.reshape(B, T, D))
    masktime = np.asarray(mask).reshape(B, T, D)[0, :, 0]
    valid_idx = np.nonzero(masktime == 0)[0][:VALID_T]
    assert valid_idx.shape[0] == VALID_T
    if valid_idx[0] == 0 and valid_idx[-1] == VALID_T - 1:
        xv = x[:, :VALID_T]                    # [B, VALID_T, D]
    else:
        xv = x[:, valid_idx]

    centers64 = np.asarray(centers, dtype=np.float64)
    ct = (2.0 * centers64).T                                # [D, K]
    cth = ct.astype(F16)
    negc2 = -(centers64 ** 2).sum(axis=1)                   # [K]
    # 3-level e5m2 bias cascade, carried in ctl8 contraction rows 509-511
    b1 = negc2.astype(E5M2)
    r1 = negc2 - b1.astype(np.float64)
    b2 = r1.astype(E5M2)
    b3 = (r1 - b2.astype(np.float64)).astype(E5M2)

    # host-side per-center softmax weight: exact residual of the device bias
    lw = negc2 - (b1.astype(np.float64) + b2.astype(np.float64)
                  + b3.astype(np.float64))
    w_host = np.exp(lw - lw.max()).astype(np.float32)       # [K], ~1.0
    _CACHE["w_host"] = w_host

    ctl = (ct - cth.astype(np.float64)).astype(E5M2)        # [D, K]
    ctl[509], ctl[510], ctl[511] = b1, b2, b3
    # const pack [P, CONST_B] bytes: ct | ctl
    parts = [
        np.ascontiguousarray(
            cth.reshape(DC, P, K).transpose(1, 0, 2)
        ).reshape(P, DC * K).view(np.uint8),
        np.ascontiguousarray(
            ctl.reshape(2, 2, P, K).transpose(2, 0, 1, 3)
        ).reshape(P, DC * K).view(np.uint8),
    ]
    cp = np.ascontiguousarray(np.concatenate(parts, axis=1))
    assert cp.shape == (P, CONST_B)
    cp = cp.ravel()

    in_maps = []
    for core in range(N_CORES):
        xc = xv[core * B_PER_CORE:(core + 1) * B_PER_CORE].reshape(ROWS, D)
        xT = np.ascontiguousarray(xc.T)                     # [D, ROWS] f32
        xh = xT.astype(F16)
        xh_p = xh.reshape(DC, P, ROWS).transpose(1, 0, 2)   # [P, DC, ROWS]
        x8 = xT.astype(E5M2)
        x8[509:512] = 1.0      # bias contraction rows (pair with ctl 509-511)
        x8_p = x8.reshape(2, 2, P, ROWS).transpose(2, 0, 1, 3)
        blocks = []
        r0 = 0
        for R in GROUPS:
            hb = np.ascontiguousarray(
                xh_p[:, :, r0:r0 + R]).reshape(P, DC * R).view(np.uint8)
            lb = np.ascontiguousarray(
                x8_p[:, :, :, r0:r0 + R]).reshape(P, DC * R).view(np.uint8)
            blocks.append(np.concatenate([hb, lb], axis=1).ravel())
            r0 += R
        xp_core = np.concatenate(blocks)
        assert xp_core.shape[0] == P * XB * ROWS
        in_maps.append({"xp": xp_core, "cp": cp})
    return in_maps


def kernel(y_pred: np.ndarray, mask: np.ndarray, centers: np.ndarray,
           **run_kwargs) -> np.ndarray:
    in_maps = prep_inputs(y_pred, mask, centers)
    nc = get_nc()
    last_err = None
    for _attempt in range(3):
        try:
            res = run_bass_kernel_spmd(nc, in_maps, core_ids=list(range(N_CORES)),
                                       **run_kwargs)
            break
        except Exception as e:  # transient NRT device errors — retry
            last_err = e
    else:
        raise last_err
    _CACHE["last_results"] = res
    e = np.concatenate(
        [np.asarray(r["out"]).reshape(B_PER_CORE, VALID_T, K)
         for r in res.results], axis=0
    ).astype(np.float32)
    ew = e * _CACHE["w_host"]
    out = ew / ew.sum(axis=-1, keepdims=True)
    return out.astype(np.float32, copy=False)
